# revision 18
# baseline (speedup 1.0000x reference)
"""Per-label whitening-coloring transform (CWCT) on 8 Trainium2 NeuronCores.

v5: symmetric fp8 gram (row-block 0 streams only its lower-triangular
129-col block; the upper-right block is recovered by one PE transpose of
the mirrored block), host-prearranged SBUF-layout DRAM tensors, iteration-1
shortcut in the diagonalization (S1 = A, M1 = K1^T via PE transposes),
per-stream PSUM tag rings, fused K-assembly via scalar_tensor_tensor,
DVE/ACT split evacuations, same-weight matmul pairing in the apply, and
staggered emission chains so group 0's apply overlaps group 1's
diagonalization.

Device math per (batch,label) group:
  * Gram + sums of content/style pixel blocks (TensorE fp8, symmetric)
  * covariance assembly (rank-1 mean correction on PE, scaling DVE/ACT)
  * M = Ltilde^{-1} via 2 quadratically-convergent diagonalization updates:
        S = V^T A V ; K = (S*(1/d)) .* (diag - striu) ; M <- K^T M ; V <- V K
  * T^T = Mc^T diag(1/sqrt(Ds*Dc)) (Ms A_s);  bias = smean - T cmean
  * apply: Y = T Xc + bias (TensorE, bf16), streamed.

Gram chunk column layout (257 wide): [ch0..127 | rowsum | ch128..255].
"""

import numpy as np
import ml_dtypes

NUM_LABELS = 8
N_CORES = 8
C = 256
RB = 2
GW = 257         # gram chunk col width: 128 low chs + sum + 128 high chs
GCH = 16         # gram chunks (128 px) per DMA tile
ACW = 2048       # apply chunk width (pixels)


def _split_waits(nc, maxw=1):
    """walrus allows only one sync-wait per instruction; hoist excess waits
    onto wait-carrying NoOps on the same engine."""
    import concourse.mybir as mybir
    n_split = 0
    for fn in nc.m.functions:
        for bb in fn.blocks:
            insts = list(bb.instructions)
            out = []
            changed = False
            for inst in insts:
                si = inst.sync_info
                if (si is not None and si.on_wait
                        and len(si.on_wait) > maxw):
                    waits = list(si.on_wait)
                    head, tail = waits[:-maxw], waits[-maxw:]
                    for ci, w in enumerate(head):
                        d = mybir.InstEventSemaphore(
                            name=f"{inst.name}-w{ci}", ins=[], outs=[])
                        d.engine = inst.engine
                        d.sync_info = mybir.SyncInfo(on_wait=[w], on_update=[])
                        nc.register_instruction(d)
                        out.append(d)
                        n_split += 1
                    si.on_wait = tail
                    changed = True
                out.append(inst)
            if changed:
                bb.instructions[:] = out
    return n_split


def _build_graph(G, Lc, Ls):
    import concourse.bass as bass
    import concourse.mybir as mybir
    import concourse.tile as tile
    from contextlib import ExitStack

    f32 = mybir.dt.float32
    bf16 = mybir.dt.bfloat16
    f8 = mybir.dt.float8e4
    MULT = mybir.AluOpType.mult

    NCHC = Lc // 128
    NCHS = Ls // 128

    nc = bass.Bass()
    xc8 = nc.declare_dram_parameter("xc8", [128, G * NCHC * GW], f8,
                                    isOutput=False)
    xs8 = nc.declare_dram_parameter("xs8", [128, G * NCHS * GW], f8,
                                    isOutput=False)
    xcn = nc.declare_dram_parameter("xcn", [128, G * 2 * Lc], bf16,
                                    isOutput=False)
    scal = nc.declare_dram_parameter("scal", [128, G * 8], f32, isOutput=False)
    cstf = nc.declare_dram_parameter("cstf", [128, 1024], f32, isOutput=False)
    cstb = nc.declare_dram_parameter("cstb", [128, 640], bf16, isOutput=False)
    yout = nc.declare_dram_parameter("y", [128, G * 2 * Lc], bf16,
                                     isOutput=True)

    with tile.TileContext(nc) as tc, ExitStack() as ctx:
        cpool = ctx.enter_context(tc.tile_pool(name="const", bufs=1))
        gxp = ctx.enter_context(tc.tile_pool(name="gx", bufs=4))
        xnp = ctx.enter_context(tc.tile_pool(name="xn", bufs=8))
        yp = ctx.enter_context(tc.tile_pool(name="y", bufs=4))
        mp = ctx.enter_context(tc.tile_pool(name="mat", bufs=1))
        sp = ctx.enter_context(tc.tile_pool(name="small", bufs=1))
        ppg = ctx.enter_context(tc.tile_pool(name="psg", bufs=1, space="PSUM"))
        ppw = ctx.enter_context(tc.tile_pool(name="psw", bufs=2, space="PSUM"))

        csf = cpool.tile([128, 1024], f32, tag="cstf", name="csf")
        nc.gpsimd.dma_start(out=csf[:, :], in_=cstf[:, :])
        csb = cpool.tile([128, 640], bf16, tag="cstb", name="csb")
        nc.gpsimd.dma_start(out=csb[:, :], in_=cstb[:, :])
        sc = cpool.tile([128, G * 8], f32, tag="scal", name="sc")
        nc.gpsimd.dma_start(out=sc[:, :], in_=scal[:, :])

        maskK = [csf[:, 0:256], csf[:, 256:512]]       # diag - striu
        diagm = [csf[:, 512:768], csf[:, 768:1024]]    # diag mask
        identb = csb[:, 512:640]                       # 128x128 I bf16

        def mm(out, lhsT, rhs, start, stop):
            nc.tensor.matmul(out, lhsT, rhs, start=start, stop=stop)

        def vcopy(dst, src):
            nc.vector.tensor_copy(dst, src)

        def scopy(dst, src):
            nc.scalar.copy(dst, src)

        cov = {}     # (g, w) -> [cov_rb0, cov_rb1] bf16
        meanc = {}   # g -> [bf16 [128,1] x2]
        means = {}   # g -> [f32 [128,1] x2]
        MDp = {}     # (g, w) -> (M, D, p_ps)
        TTb = {}     # g -> (TT, bias)

        def emit_gram(g, w):
            """Symmetric fp8 gram for (g, w). Returns open psums [ps0, ps1].

            ps0 [128,129]: rows ch0-127 x [ch0-127 | sum]
            ps1 [128,257]: rows ch128-255 x [ch0-127 | sum | ch128-255]
            """
            src = xc8 if w == "c" else xs8
            NCH = NCHC if w == "c" else NCHS
            ps0 = ppg.tile([128, 129], f32, tag=f"g{w}0", name="g0")
            ps1 = ppg.tile([128, GW], f32, tag=f"g{w}1", name="g1")
            done = 0
            while done < NCH:
                # small first load so the PE starts as early as possible
                wch = min(4 if done == 0 and g == 0 and w == "c" else GCH,
                          NCH - done)
                xt = gxp.tile([128, GCH * GW], f8, tag="gx", name="gx")
                base = (g * NCH + done) * GW
                nc.sync.dma_start(
                    out=xt[:, 0:wch * GW],
                    in_=src[:, base:base + wch * GW])
                for ci in range(wch):
                    jb = ci * GW
                    first = (done == 0 and ci == 0)
                    mm(ps0[:, :], xt[:, jb:jb + 128], xt[:, jb:jb + 129],
                       start=first, stop=False)
                    mm(ps1[:, :], xt[:, jb + 129:jb + 257],
                       xt[:, jb:jb + 257], start=first, stop=False)
                done += wch
            return [ps0, ps1]

        def emit_corr(g, w, ps):
            """Rank-1 mean correction + covariance evac for stream (g, w)."""
            ia = 4 if w == "c" else 5       # 1/a index in scal
            na = 1 if w == "c" else 3       # -1/a index
            iam1 = 0 if w == "c" else 2     # 1/(a-1) index
            wtag = f"w{w}"
            s_col = [sp.tile([128, 1], bf16, tag=f"scol{w}{rb}",
                             name=f"scol{w}{rb}", bufs=2) for rb in range(RB)]
            vcopy(s_col[0][:, :], ps[0][:, 128:129])
            scopy(s_col[1][:, :], ps[1][:, 128:129])
            # means
            if w == "c":
                mc = [sp.tile([128, 1], bf16, tag=f"mc{rb}", name=f"mc{rb}",
                              bufs=2) for rb in range(RB)]
                nc.vector.tensor_scalar_mul(
                    mc[0][:, :], s_col[0][:, :],
                    sc[:, g * 8 + ia:g * 8 + ia + 1])
                nc.scalar.mul(mc[1][:, :], s_col[1][:, :],
                              sc[:, g * 8 + ia:g * 8 + ia + 1])
                meanc[g] = mc
            else:
                msn = [sp.tile([128, 1], f32, tag=f"ms{rb}", name=f"ms{rb}",
                               bufs=2) for rb in range(RB)]
                nc.vector.tensor_scalar_mul(
                    msn[0][:, :], s_col[0][:, :],
                    sc[:, g * 8 + ia:g * 8 + ia + 1])
                nc.scalar.mul(msn[1][:, :], s_col[1][:, :],
                              sc[:, g * 8 + ia:g * 8 + ia + 1])
                means[g] = msn
            # sT row [1, 257] in chunk layout [s_low | 0 | s_high]
            sT = sp.tile([1, GW], bf16, tag=f"sT{w}", name=f"sT{w}", bufs=2)
            sTs = sp.tile([1, GW], bf16, tag=f"sTs{w}", name=f"sTs{w}",
                          bufs=2)
            nc.vector.memset(sT[0:1, 128:129], 0.0)
            for rb in range(RB):
                tp = ppw.tile([1, 128], bf16, tag=wtag, name="tp")
                nc.tensor.transpose(tp[0:1, :], s_col[rb][:, 0:1], identb)
                off = 0 if rb == 0 else 129
                vcopy(sT[0:1, off:off + 128], tp[0:1, :])
            nc.vector.tensor_scalar_mul(
                sTs[0:1, :], sT[0:1, :], sc[0:1, g * 8 + na:g * 8 + na + 1])
            mm(ps[0][:, :], sTs[0:1, 0:128], sT[0:1, 0:129],
               start=False, stop=True)
            mm(ps[1][:, :], sTs[0:1, 129:257], sT[0:1, 0:257],
               start=False, stop=True)
            # covariance evac; cov1 first (its low block feeds cov0's high)
            cv = [mp.tile([128, 256], bf16, tag=f"cov{w}{rb}",
                          name=f"cov{w}{rb}", bufs=2) for rb in range(RB)]
            sca = sc[:, g * 8 + iam1:g * 8 + iam1 + 1]
            nc.vector.tensor_scalar_mul(cv[1][:, 0:128], ps[1][:, 0:128], sca)
            nc.scalar.mul(cv[1][:, 128:256], ps[1][:, 129:257], sca)
            nc.scalar.mul(cv[0][:, 0:128], ps[0][:, 0:128], sca)
            tp2 = ppw.tile([128, 128], bf16, tag=wtag, name="tp2")
            nc.tensor.transpose(tp2[:, :], cv[1][:, 0:128], identb)
            vcopy(cv[0][:, 128:256], tp2[:, :])
            cov[(g, w)] = cv

        def diag_gen(g, w):
            """Diagonalization iteration; yields between engine phases."""
            A = cov[(g, w)]
            p = f"{w}{g % 2}"
            wtag = f"w{w}"

            def make_K(S_tiles, from_psum):
                K = [mp.tile([128, 256], bf16, tag=f"{p}K{rb}",
                             name=f"{p}K{rb}", bufs=2) for rb in range(RB)]
                for rb in range(RB):
                    d = sp.tile([128, 1], f32, tag=f"{p}d{rb}",
                                name=f"{p}d{rb}", bufs=2)
                    rd = sp.tile([128, 1], f32, tag=f"{p}rd{rb}",
                                 name=f"{p}rd{rb}", bufs=2)
                    junk = mp.tile([128, 256], bf16, tag="junk", name="junk",
                                   bufs=4)
                    nc.vector.scalar_tensor_tensor(
                        junk[:, :], S_tiles[rb][:, :], 1.0, diagm[rb],
                        MULT, MULT, accum_out=d[:, :])
                    nc.vector.reciprocal(rd[:, :], d[:, :])
                    nc.vector.scalar_tensor_tensor(
                        K[rb][:, :], S_tiles[rb][:, :], rd[:, :], maskK[rb],
                        MULT, MULT)
                return K

            # ---- iteration 1 shortcut: S1 = A, V1 = K1, M1 = K1^T ----
            K = make_K(A, False)
            V = K
            yield
            t_ps = [ppw.tile([128, 256], bf16, tag=wtag, name="t")
                    for _ in range(RB)]
            for rb in range(RB):
                for kc in range(RB):
                    nc.tensor.transpose(t_ps[rb][:, kc * 128:(kc + 1) * 128],
                                        K[kc][:, rb * 128:(rb + 1) * 128],
                                        identb)
            M = [mp.tile([128, 256], bf16, tag=f"{p}M{rb}", name=f"{p}M{rb}",
                         bufs=2) for rb in range(RB)]
            vcopy(M[0][:, :], t_ps[0][:, :])
            scopy(M[1][:, :], t_ps[1][:, :])
            yield
            # ---- iteration 2 (full) ----
            av_ps = [ppw.tile([128, 256], f32, tag=wtag, name="w")
                     for _ in range(RB)]
            for rb in range(RB):
                for kc in range(RB):
                    mm(av_ps[rb][:, :], A[kc][:, rb * 128:(rb + 1) * 128],
                       V[kc][:, :], start=(kc == 0), stop=(kc == RB - 1))
            av = [mp.tile([128, 256], bf16, tag=f"{p}av{rb}",
                          name=f"{p}av{rb}", bufs=1) for rb in range(RB)]
            vcopy(av[0][:, :], av_ps[0][:, :])
            scopy(av[1][:, :], av_ps[1][:, :])
            yield
            s_ps = [ppw.tile([128, 256], f32, tag=wtag, name="w")
                    for _ in range(RB)]
            for rb in range(RB):
                for kc in range(RB):
                    mm(s_ps[rb][:, :], V[kc][:, rb * 128:(rb + 1) * 128],
                       av[kc][:, :], start=(kc == 0), stop=(kc == RB - 1))
            yield
            K2 = make_K(s_ps, True)
            yield
            m_ps = [ppw.tile([128, 256], f32, tag=wtag, name="w")
                    for _ in range(RB)]
            for rb in range(RB):
                for kc in range(RB):
                    mm(m_ps[rb][:, :], K2[kc][:, rb * 128:(rb + 1) * 128],
                       M[kc][:, :], start=(kc == 0), stop=(kc == RB - 1))
            v_ps = [ppw.tile([128, 256], f32, tag=wtag, name="w")
                    for _ in range(RB)]
            for rb in range(RB):
                for kc in range(RB):
                    mm(v_ps[rb][:, :], M[kc][:, rb * 128:(rb + 1) * 128],
                       K2[kc][:, :], start=(kc == 0), stop=(kc == RB - 1))
            M = [mp.tile([128, 256], bf16, tag=f"{p}M{rb}", name=f"{p}M{rb}",
                         bufs=2) for rb in range(RB)]
            V = [mp.tile([128, 256], bf16, tag=f"{p}V{rb}", name=f"{p}V{rb}",
                         bufs=2) for rb in range(RB)]
            vcopy(M[0][:, :], m_ps[0][:, :])
            scopy(M[1][:, :], m_ps[1][:, :])
            scopy(V[0][:, :], v_ps[0][:, :])
            vcopy(V[1][:, :], v_ps[1][:, :])
            yield
            p_ps = [ppw.tile([128, 256], f32, tag=wtag, name="w")
                    for _ in range(RB)]
            for rb in range(RB):
                for kc in range(RB):
                    mm(p_ps[rb][:, :], V[kc][:, rb * 128:(rb + 1) * 128],
                       A[kc][:, :], start=(kc == 0), stop=(kc == RB - 1))
            D = [sp.tile([128, 1], f32, tag=f"{p}D{rb}", name=f"{p}D{rb}",
                         bufs=1) for rb in range(RB)]
            for rb in range(RB):
                junk = mp.tile([128, 256], bf16, tag="junk", name="junk",
                               bufs=4)
                nc.vector.scalar_tensor_tensor(
                    junk[:, :], p_ps[rb][:, :], 1.0, M[rb][:, :],
                    MULT, MULT, accum_out=D[rb][:, :])
            yield
            MDp[(g, w)] = (M, D, p_ps)

        def group_gen(g):
            gens = [diag_gen(g, "c"), diag_gen(g, "s")]
            alive = [True, True]
            while any(alive):
                for i, gi in enumerate(gens):
                    if alive[i]:
                        try:
                            next(gi)
                        except StopIteration:
                            alive[i] = False
                yield
            Mc, Dc, pc_ps = MDp[(g, "c")]
            Ms, Ds, ps_ps = MDp[(g, "s")]
            # comb = 1/sqrt(Ds*Dc); LsT = comb * (Ms A_s) ; T^T = Mc^T LsT
            LsT = [mp.tile([128, 256], bf16, tag=f"LsT{g % 2}{rb}",
                           name=f"LsT{rb}", bufs=1) for rb in range(RB)]
            for rb in range(RB):
                comb = sp.tile([128, 1], f32, tag=f"comb{g % 2}{rb}",
                               name=f"comb{rb}", bufs=1)
                nc.vector.tensor_mul(comb[:, :], Ds[rb][:, :], Dc[rb][:, :])
                nc.scalar.sqrt(comb[:, :], comb[:, :])
                nc.vector.reciprocal(comb[:, :], comb[:, :])
                if rb == 0:
                    nc.vector.tensor_scalar_mul(LsT[rb][:, :],
                                                ps_ps[rb][:, :], comb[:, :])
                else:
                    nc.scalar.mul(LsT[rb][:, :], ps_ps[rb][:, :], comb[:, :])
            yield
            TT = [mp.tile([128, 256], bf16, tag=f"TT{g % 2}{rb}",
                          name=f"TT{rb}", bufs=1) for rb in range(RB)]
            tt_ps = [ppg.tile([128, 256], f32, tag=f"gc{rb}", name="tt")
                     for rb in range(RB)]
            for rb in range(RB):
                for kc in range(RB):
                    mm(tt_ps[rb][:, :], Mc[kc][:, rb * 128:(rb + 1) * 128],
                       LsT[kc][:, :], start=(kc == 0), stop=(kc == RB - 1))
            vcopy(TT[0][:, :], tt_ps[0][:, :])
            scopy(TT[1][:, :], tt_ps[1][:, :])
            yield
            bias = [sp.tile([128, 1], f32, tag=f"bias{g % 2}{rb}",
                            name=f"bias{rb}", bufs=1) for rb in range(RB)]
            for rb in range(RB):
                b_ps = ppw.tile([128, 1], f32, tag="ws", name="b_ps")
                for kc in range(RB):
                    mm(b_ps[:, :], TT[kc][:, rb * 128:(rb + 1) * 128],
                       meanc[g][kc][:, :], start=(kc == 0), stop=(kc == RB - 1))
                nc.vector.tensor_sub(bias[rb][:, :], means[g][rb][:, :],
                                     b_ps[:, :])
            yield
            TTb[g] = (TT, bias)

        aps_tags = ["gc0", "gc1", "gs0", "gs1"]
        aps_ctr = [0]

        def apply_gen(g):
            TT, bias = TTb[g]
            # taper chunk sizes so the final store drain is short
            sizes = []
            rem = Lc
            while rem > 0:
                if rem > 3 * ACW:
                    take = ACW
                elif rem > 3 * (ACW // 2):
                    take = ACW // 2
                else:
                    take = min(rem, max(512, ACW // 4))
                sizes.append(take)
                rem -= take
            chunks = []
            c0 = 0
            for take in sizes:
                chunks.append((c0, take))
                c0 += take
            for (c0, cwc) in chunks:
                xn = xnp.tile([128, 2 * ACW], bf16, tag="xn", name="xn")
                for rb in range(RB):
                    nc.sync.dma_start(
                        out=xn[:, rb * cwc:rb * cwc + cwc],
                        in_=xcn[:, g * 2 * Lc + rb * Lc + c0:
                                g * 2 * Lc + rb * Lc + c0 + cwc])
                y2 = yp.tile([128, 2 * ACW], bf16, tag="y2", name="y2")
                nh = cwc // 512
                for rb in range(RB):
                    for h0 in range(0, nh, 2):
                        hs = [h for h in (h0, h0 + 1) if h < nh]
                        pss = []
                        for h in hs:
                            y_ps = ppg.tile(
                                [128, 512], f32,
                                tag=aps_tags[aps_ctr[0] % 4], name="y")
                            aps_ctr[0] += 1
                            pss.append(y_ps)
                        # same stationary weight drives both h-blocks
                        for kc in range(RB):
                            for hi, h in enumerate(hs):
                                mm(pss[hi][:, :],
                                   TT[kc][:, rb * 128:(rb + 1) * 128],
                                   xn[:, kc * cwc + h * 512:
                                      kc * cwc + h * 512 + 512],
                                   start=(kc == 0), stop=(kc == RB - 1))
                        for hi, h in enumerate(hs):
                            dst = y2[:, rb * cwc + h * 512:
                                     rb * cwc + h * 512 + 512]
                            nc.vector.tensor_scalar_add(dst[:, 0:256],
                                                        pss[hi][:, 0:256],
                                                        bias[rb][:, :])
                            nc.scalar.add(dst[:, 256:512],
                                          pss[hi][:, 256:512],
                                          add=bias[rb][:, :])
                        yield
                    # store this rb half as soon as its evacs complete
                    nc.gpsimd.dma_start(
                        out=yout[:, g * 2 * Lc + rb * Lc + c0:
                                 g * 2 * Lc + rb * Lc + c0 + cwc],
                        in_=y2[:, rb * cwc:rb * cwc + cwc])
            yield

        # ---------------- emission schedule ----------------
        for g in range(G):
            for w in ("c", "s"):
                ps = emit_gram(g, w)
                emit_corr(g, w, ps)

        def chain(g):
            yield from group_gen(g)
            yield from apply_gen(g)

        chains = [chain(g) for g in range(G)]
        calive = [True] * G
        # stagger: earlier chains advance twice per later chain's step so
        # group g's apply emission (and scheduler priority) lands ahead of
        # group g+1's diagonalization.
        weights = [2 if g < G - 1 else 1 for g in range(G)]
        while any(calive):
            for i, ci in enumerate(chains):
                for _ in range(weights[i] if any(calive[i + 1:]) else 1):
                    if calive[i]:
                        try:
                            next(ci)
                        except StopIteration:
                            calive[i] = False
    _split_waits(nc)
    return nc


def _consts_np():
    i = np.arange(128)
    f = np.arange(256)
    cstf = np.zeros((128, 1024), dtype=np.float32)
    for rb in range(RB):
        diag = (f[None, :] == (i[:, None] + rb * 128)).astype(np.float32)
        striu = (f[None, :] > (i[:, None] + rb * 128)).astype(np.float32)
        cstf[:, rb * 256:(rb + 1) * 256] = diag - striu
        cstf[:, 512 + rb * 256:512 + (rb + 1) * 256] = diag
    cstb = np.zeros((128, 640), dtype=ml_dtypes.bfloat16)
    for rb in range(RB):
        cstb[:, rb * 256:(rb + 1) * 256] = (
            f[None, :] == (i[:, None] + rb * 128))
    cstb[:, 512:640] = (i[None, :] == i[:, None])
    return cstf, cstb


def kernel(c_feat, s_feat, c_mask, s_mask, _trace=False, _result_box=None):
    from concourse.bass_utils import run_bass_kernel_spmd

    f8 = ml_dtypes.float8_e4m3fn
    bf16 = ml_dtypes.bfloat16

    c_feat = np.ascontiguousarray(np.asarray(c_feat, dtype=np.float32))
    s_feat = np.ascontiguousarray(np.asarray(s_feat, dtype=np.float32))
    cm = np.asarray(c_mask).reshape(-1)
    sm = np.asarray(s_mask).reshape(-1)

    B, Cc, H, W = c_feat.shape
    assert Cc == C
    N = H * W
    cf = c_feat.reshape(B, C, N)
    sf = s_feat.reshape(B, C, N)

    cnt_a = np.bincount(cm, minlength=NUM_LABELS).astype(np.float64)
    cnt_b = np.bincount(sm, minlength=NUM_LABELS).astype(np.float64)
    guide = ((cnt_a > 10) & (cnt_b > 10) & (cnt_a < 10 * cnt_b)
             & (cnt_b < 10 * cnt_a))
    glabels = [l for l in range(NUM_LABELS) if guide[l]]

    out = cf.copy()
    if not glabels:
        return out.reshape(B, C, H, W)

    idx_c = {l: np.nonzero(cm == l)[0] for l in glabels}
    idx_s = {l: np.nonzero(sm == l)[0] for l in glabels}

    groups = [(bb, l) for l in glabels for bb in range(B)]
    n_real = len(groups)
    G = max(1, (n_real + N_CORES - 1) // N_CORES)
    while len(groups) < N_CORES * G:
        groups.append(groups[0])

    def rnd(x, m):
        return ((int(x) + m - 1) // m) * m

    Lc = rnd(max(len(idx_c[l]) for l in glabels), 512)
    Ls = rnd(max(len(idx_s[l]) for l in glabels), 512)
    NCHC = Lc // 128
    NCHS = Ls // 128

    cstf, cstb = _consts_np()
    in_maps = []
    for core in range(N_CORES):
        xc8 = np.zeros((128, G * NCHC * GW), dtype=f8)
        xs8 = np.zeros((128, G * NCHS * GW), dtype=f8)
        xcn = np.zeros((128, G * 2 * Lc), dtype=bf16)
        scal = np.zeros((128, G * 8), dtype=np.float32)
        for g in range(G):
            bb, l = groups[core * G + g]
            ic, isx = idx_c[l], idx_s[l]
            a, b = float(len(ic)), float(len(isx))
            xc = cf[bb][:, ic]                      # [C, a]
            xs = sf[bb][:, isx]
            for (x, L, NCH, dst) in ((xc, Lc, NCHC, xc8), (xs, Ls, NCHS, xs8)):
                n = x.shape[1]
                arr = np.zeros((L, GW), dtype=np.float32)
                arr[:n, 0:128] = x.T[:, 0:128]
                arr[:n, 128] = 1.0
                arr[:n, 129:257] = x.T[:, 128:256]
                t = arr.reshape(NCH, 128, GW).transpose(1, 0, 2)
                dst[:, g * NCH * GW:(g + 1) * NCH * GW] = (
                    t.reshape(128, NCH * GW).astype(f8))
            arr = np.zeros((256, Lc), dtype=np.float32)
            arr[:, :len(ic)] = xc
            xcn[:, g * 2 * Lc:g * 2 * Lc + Lc] = arr[0:128].astype(bf16)
            xcn[:, g * 2 * Lc + Lc:(g + 1) * 2 * Lc] = arr[128:256].astype(bf16)
            vals = [1.0 / (a - 1.0), -1.0 / a, 1.0 / (b - 1.0), -1.0 / b,
                    1.0 / a, 1.0 / b, 0.0, 0.0]
            scal[:, g * 8:(g + 1) * 8] = np.asarray(vals,
                                                    dtype=np.float32)[None, :]
        in_maps.append({"xc8": xc8, "xs8": xs8, "xcn": xcn, "scal": scal,
                        "cstf": cstf, "cstb": cstb})

    nc = _build_graph(G, Lc, Ls)
    res = run_bass_kernel_spmd(nc, in_maps, core_ids=list(range(N_CORES)),
                               trace=_trace)
    if _result_box is not None:
        _result_box.append(res)

    for core in range(N_CORES):
        y = np.asarray(res.results[core]["y"], dtype=np.float32)
        for g in range(G):
            slot = core * G + g
            if slot >= n_real:
                continue
            bb, l = groups[slot]
            ic = idx_c[l]
            na = len(ic)
            out[bb][0:128, ic] = y[:, g * 2 * Lc:g * 2 * Lc + na]
            out[bb][128:256, ic] = y[:, g * 2 * Lc + Lc:g * 2 * Lc + Lc + na]

    return out.reshape(B, C, H, W)


# revision 19
# speedup vs baseline: 1.0806x; 1.0806x over previous
"""Per-label whitening-coloring transform (CWCT) on 8 Trainium2 NeuronCores.

v5: symmetric fp8 gram (row-block 0 streams only its lower-triangular
129-col block; the upper-right block is recovered by one PE transpose of
the mirrored block), host-prearranged SBUF-layout DRAM tensors, iteration-1
shortcut in the diagonalization (S1 = A, M1 = K1^T via PE transposes),
per-stream PSUM tag rings, fused K-assembly via scalar_tensor_tensor,
DVE/ACT split evacuations, same-weight matmul pairing in the apply, and
staggered emission chains so group 0's apply overlaps group 1's
diagonalization.

Device math per (batch,label) group:
  * Gram + sums of content/style pixel blocks (TensorE fp8, symmetric)
  * covariance assembly (rank-1 mean correction on PE, scaling DVE/ACT)
  * M = Ltilde^{-1} via 2 quadratically-convergent diagonalization updates:
        S = V^T A V ; K = (S*(1/d)) .* (diag - striu) ; M <- K^T M ; V <- V K
  * T^T = Mc^T diag(1/sqrt(Ds*Dc)) (Ms A_s);  bias = smean - T cmean
  * apply: Y = T Xc + bias (TensorE, bf16), streamed.

Gram chunk column layout (257 wide): [ch0..127 | rowsum | ch128..255].
"""

import numpy as np
import ml_dtypes

NUM_LABELS = 8
N_CORES = 8
C = 256
RB = 2
GW = 257         # gram chunk col width: 128 low chs + sum + 128 high chs
GCH = 16         # gram chunks (128 px) per DMA tile
ACW = 2048       # apply chunk width (pixels)


def _split_waits(nc, maxw=1):
    """walrus allows only one sync-wait per instruction; hoist excess waits
    onto wait-carrying NoOps on the same engine."""
    import concourse.mybir as mybir
    n_split = 0
    for fn in nc.m.functions:
        for bb in fn.blocks:
            insts = list(bb.instructions)
            out = []
            changed = False
            for inst in insts:
                si = inst.sync_info
                if (si is not None and si.on_wait
                        and len(si.on_wait) > maxw):
                    waits = list(si.on_wait)
                    head, tail = waits[:-maxw], waits[-maxw:]
                    for ci, w in enumerate(head):
                        d = mybir.InstEventSemaphore(
                            name=f"{inst.name}-w{ci}", ins=[], outs=[])
                        d.engine = inst.engine
                        d.sync_info = mybir.SyncInfo(on_wait=[w], on_update=[])
                        nc.register_instruction(d)
                        out.append(d)
                        n_split += 1
                    si.on_wait = tail
                    changed = True
                out.append(inst)
            if changed:
                bb.instructions[:] = out
    return n_split


def _build_graph(G, Lc, Ls):
    import concourse.bass as bass
    import concourse.mybir as mybir
    import concourse.tile as tile
    from contextlib import ExitStack

    f32 = mybir.dt.float32
    bf16 = mybir.dt.bfloat16
    f8 = mybir.dt.float8e4
    MULT = mybir.AluOpType.mult

    NCHC = Lc // 128
    NCHS = Ls // 128

    nc = bass.Bass()
    xc8 = nc.declare_dram_parameter("xc8", [128, G * NCHC * GW], f8,
                                    isOutput=False)
    xs8 = nc.declare_dram_parameter("xs8", [128, G * NCHS * GW], f8,
                                    isOutput=False)
    xcn = nc.declare_dram_parameter("xcn", [128, G * 2 * Lc], bf16,
                                    isOutput=False)
    scal = nc.declare_dram_parameter("scal", [128, G * 8], f32, isOutput=False)
    cstf = nc.declare_dram_parameter("cstf", [128, 1024], f32, isOutput=False)
    cstb = nc.declare_dram_parameter("cstb", [128, 640], bf16, isOutput=False)
    yout = nc.declare_dram_parameter("y", [128, G * 2 * Lc], bf16,
                                     isOutput=True)

    with tile.TileContext(nc) as tc, ExitStack() as ctx:
        cpool = ctx.enter_context(tc.tile_pool(name="const", bufs=1))
        gxp = ctx.enter_context(tc.tile_pool(name="gx", bufs=4))
        xnp = ctx.enter_context(tc.tile_pool(name="xn", bufs=8))
        yp = ctx.enter_context(tc.tile_pool(name="y", bufs=4))
        mp = ctx.enter_context(tc.tile_pool(name="mat", bufs=1))
        sp = ctx.enter_context(tc.tile_pool(name="small", bufs=1))
        ppg = ctx.enter_context(tc.tile_pool(name="psg", bufs=1, space="PSUM"))
        ppw = ctx.enter_context(tc.tile_pool(name="psw", bufs=2, space="PSUM"))

        csf = cpool.tile([128, 1024], f32, tag="cstf", name="csf")
        nc.gpsimd.dma_start(out=csf[:, :], in_=cstf[:, :])
        csb = cpool.tile([128, 640], bf16, tag="cstb", name="csb")
        nc.gpsimd.dma_start(out=csb[:, :], in_=cstb[:, :])
        sc = cpool.tile([128, G * 8], f32, tag="scal", name="sc")
        nc.gpsimd.dma_start(out=sc[:, :], in_=scal[:, :])

        maskK = [csf[:, 0:256], csf[:, 256:512]]       # diag - striu
        diagm = [csf[:, 512:768], csf[:, 768:1024]]    # diag mask
        identb = csb[:, 512:640]                       # 128x128 I bf16

        def mm(out, lhsT, rhs, start, stop):
            nc.tensor.matmul(out, lhsT, rhs, start=start, stop=stop)

        def vcopy(dst, src):
            nc.vector.tensor_copy(dst, src)

        def scopy(dst, src):
            nc.scalar.copy(dst, src)

        cov = {}     # (g, w) -> [cov_rb0, cov_rb1] bf16
        meanc = {}   # g -> [bf16 [128,1] x2]
        means = {}   # g -> [f32 [128,1] x2]
        MDp = {}     # (g, w) -> (M, D, p_ps)
        TTb = {}     # g -> (TT, bias)

        def emit_gram(g, w):
            """Symmetric fp8 gram for (g, w). Returns open psums [ps0, ps1].

            ps0 [128,129]: rows ch0-127 x [ch0-127 | sum]
            ps1 [128,257]: rows ch128-255 x [ch0-127 | sum | ch128-255]
            """
            src = xc8 if w == "c" else xs8
            NCH = NCHC if w == "c" else NCHS
            ps0 = ppg.tile([128, 129], f32, tag=f"g{w}0", name="g0")
            ps1 = ppg.tile([128, GW], f32, tag=f"g{w}1", name="g1")
            done = 0
            while done < NCH:
                # small first load so the PE starts as early as possible
                wch = min(4 if done == 0 and g == 0 and w == "c" else GCH,
                          NCH - done)
                xt = gxp.tile([128, GCH * GW], f8, tag="gx", name="gx")
                base = (g * NCH + done) * GW
                nc.sync.dma_start(
                    out=xt[:, 0:wch * GW],
                    in_=src[:, base:base + wch * GW])
                for ci in range(wch):
                    jb = ci * GW
                    first = (done == 0 and ci == 0)
                    mm(ps0[:, :], xt[:, jb:jb + 128], xt[:, jb:jb + 129],
                       start=first, stop=False)
                    mm(ps1[:, :], xt[:, jb + 129:jb + 257],
                       xt[:, jb:jb + 257], start=first, stop=False)
                done += wch
            return [ps0, ps1]

        def emit_corr(g, w, ps):
            """Rank-1 mean correction + covariance evac for stream (g, w)."""
            ia = 4 if w == "c" else 5       # 1/a index in scal
            na = 1 if w == "c" else 3       # -1/a index
            iam1 = 0 if w == "c" else 2     # 1/(a-1) index
            wtag = f"w{w}"
            s_col = [sp.tile([128, 1], bf16, tag=f"scol{w}{rb}",
                             name=f"scol{w}{rb}", bufs=2) for rb in range(RB)]
            vcopy(s_col[0][:, :], ps[0][:, 128:129])
            scopy(s_col[1][:, :], ps[1][:, 128:129])
            # means
            if w == "c":
                mc = [sp.tile([128, 1], bf16, tag=f"mc{rb}", name=f"mc{rb}",
                              bufs=2) for rb in range(RB)]
                nc.vector.tensor_scalar_mul(
                    mc[0][:, :], s_col[0][:, :],
                    sc[:, g * 8 + ia:g * 8 + ia + 1])
                nc.scalar.mul(mc[1][:, :], s_col[1][:, :],
                              sc[:, g * 8 + ia:g * 8 + ia + 1])
                meanc[g] = mc
            else:
                msn = [sp.tile([128, 1], f32, tag=f"ms{rb}", name=f"ms{rb}",
                               bufs=2) for rb in range(RB)]
                nc.vector.tensor_scalar_mul(
                    msn[0][:, :], s_col[0][:, :],
                    sc[:, g * 8 + ia:g * 8 + ia + 1])
                nc.scalar.mul(msn[1][:, :], s_col[1][:, :],
                              sc[:, g * 8 + ia:g * 8 + ia + 1])
                means[g] = msn
            # sT row [1, 257] in chunk layout [s_low | 0 | s_high]
            sT = sp.tile([1, GW], bf16, tag=f"sT{w}", name=f"sT{w}", bufs=2)
            sTs = sp.tile([1, GW], bf16, tag=f"sTs{w}", name=f"sTs{w}",
                          bufs=2)
            nc.vector.memset(sT[0:1, 128:129], 0.0)
            for rb in range(RB):
                tp = ppw.tile([1, 128], bf16, tag=wtag, name="tp")
                nc.tensor.transpose(tp[0:1, :], s_col[rb][:, 0:1], identb)
                off = 0 if rb == 0 else 129
                vcopy(sT[0:1, off:off + 128], tp[0:1, :])
            nc.vector.tensor_scalar_mul(
                sTs[0:1, :], sT[0:1, :], sc[0:1, g * 8 + na:g * 8 + na + 1])
            mm(ps[0][:, :], sTs[0:1, 0:128], sT[0:1, 0:129],
               start=False, stop=True)
            mm(ps[1][:, :], sTs[0:1, 129:257], sT[0:1, 0:257],
               start=False, stop=True)
            # covariance evac; cov1 first (its low block feeds cov0's high)
            cv = [mp.tile([128, 256], bf16, tag=f"cov{w}{rb}",
                          name=f"cov{w}{rb}", bufs=2) for rb in range(RB)]
            sca = sc[:, g * 8 + iam1:g * 8 + iam1 + 1]
            nc.vector.tensor_scalar_mul(cv[1][:, 0:128], ps[1][:, 0:128], sca)
            nc.scalar.mul(cv[1][:, 128:256], ps[1][:, 129:257], sca)
            nc.scalar.mul(cv[0][:, 0:128], ps[0][:, 0:128], sca)
            tp2 = ppw.tile([128, 128], bf16, tag=wtag, name="tp2")
            nc.tensor.transpose(tp2[:, :], cv[1][:, 0:128], identb)
            vcopy(cv[0][:, 128:256], tp2[:, :])
            cov[(g, w)] = cv

        def diag_gen(g, w):
            """Diagonalization iteration; yields between engine phases."""
            A = cov[(g, w)]
            p = f"{w}{g % 2}"
            wtag = f"w{w}"

            def make_K(S_tiles, from_psum):
                K = [mp.tile([128, 256], bf16, tag=f"{p}K{rb}",
                             name=f"{p}K{rb}", bufs=2) for rb in range(RB)]
                for rb in range(RB):
                    d = sp.tile([128, 1], f32, tag=f"{p}d{rb}",
                                name=f"{p}d{rb}", bufs=2)
                    rd = sp.tile([128, 1], f32, tag=f"{p}rd{rb}",
                                 name=f"{p}rd{rb}", bufs=2)
                    junk = mp.tile([128, 256], bf16, tag="junk", name="junk",
                                   bufs=4)
                    nc.vector.scalar_tensor_tensor(
                        junk[:, :], S_tiles[rb][:, :], 1.0, diagm[rb],
                        MULT, MULT, accum_out=d[:, :])
                    nc.vector.reciprocal(rd[:, :], d[:, :])
                    nc.vector.scalar_tensor_tensor(
                        K[rb][:, :], S_tiles[rb][:, :], rd[:, :], maskK[rb],
                        MULT, MULT)
                return K

            # ---- iteration 1 shortcut: S1 = A, V1 = K1, M1 = K1^T ----
            K = make_K(A, False)
            V = K
            yield
            t_ps = [ppw.tile([128, 256], bf16, tag=wtag, name="t")
                    for _ in range(RB)]
            for rb in range(RB):
                for kc in range(RB):
                    nc.tensor.transpose(t_ps[rb][:, kc * 128:(kc + 1) * 128],
                                        K[kc][:, rb * 128:(rb + 1) * 128],
                                        identb)
            M = [mp.tile([128, 256], bf16, tag=f"{p}M{rb}", name=f"{p}M{rb}",
                         bufs=2) for rb in range(RB)]
            vcopy(M[0][:, :], t_ps[0][:, :])
            scopy(M[1][:, :], t_ps[1][:, :])
            yield
            # ---- iteration 2 (full) ----
            av_ps = [ppw.tile([128, 256], f32, tag=wtag, name="w")
                     for _ in range(RB)]
            for rb in range(RB):
                for kc in range(RB):
                    mm(av_ps[rb][:, :], A[kc][:, rb * 128:(rb + 1) * 128],
                       V[kc][:, :], start=(kc == 0), stop=(kc == RB - 1))
            av = [mp.tile([128, 256], bf16, tag=f"{p}av{rb}",
                          name=f"{p}av{rb}", bufs=1) for rb in range(RB)]
            vcopy(av[0][:, :], av_ps[0][:, :])
            scopy(av[1][:, :], av_ps[1][:, :])
            yield
            s_ps = [ppw.tile([128, 256], f32, tag=wtag, name="w")
                    for _ in range(RB)]
            for rb in range(RB):
                for kc in range(RB):
                    mm(s_ps[rb][:, :], V[kc][:, rb * 128:(rb + 1) * 128],
                       av[kc][:, :], start=(kc == 0), stop=(kc == RB - 1))
            yield
            K2 = make_K(s_ps, True)
            yield
            m_ps = [ppw.tile([128, 256], f32, tag=wtag, name="w")
                    for _ in range(RB)]
            for rb in range(RB):
                for kc in range(RB):
                    mm(m_ps[rb][:, :], K2[kc][:, rb * 128:(rb + 1) * 128],
                       M[kc][:, :], start=(kc == 0), stop=(kc == RB - 1))
            v_ps = [ppw.tile([128, 256], f32, tag=wtag, name="w")
                    for _ in range(RB)]
            for rb in range(RB):
                for kc in range(RB):
                    mm(v_ps[rb][:, :], M[kc][:, rb * 128:(rb + 1) * 128],
                       K2[kc][:, :], start=(kc == 0), stop=(kc == RB - 1))
            M = [mp.tile([128, 256], bf16, tag=f"{p}M{rb}", name=f"{p}M{rb}",
                         bufs=2) for rb in range(RB)]
            V = [mp.tile([128, 256], bf16, tag=f"{p}V{rb}", name=f"{p}V{rb}",
                         bufs=2) for rb in range(RB)]
            vcopy(M[0][:, :], m_ps[0][:, :])
            scopy(M[1][:, :], m_ps[1][:, :])
            scopy(V[0][:, :], v_ps[0][:, :])
            vcopy(V[1][:, :], v_ps[1][:, :])
            yield
            p_ps = [ppw.tile([128, 256], f32, tag=wtag, name="w")
                    for _ in range(RB)]
            for rb in range(RB):
                for kc in range(RB):
                    mm(p_ps[rb][:, :], V[kc][:, rb * 128:(rb + 1) * 128],
                       A[kc][:, :], start=(kc == 0), stop=(kc == RB - 1))
            D = [sp.tile([128, 1], f32, tag=f"{p}D{rb}", name=f"{p}D{rb}",
                         bufs=1) for rb in range(RB)]
            for rb in range(RB):
                junk = mp.tile([128, 256], bf16, tag="junk", name="junk",
                               bufs=4)
                nc.vector.scalar_tensor_tensor(
                    junk[:, :], p_ps[rb][:, :], 1.0, M[rb][:, :],
                    MULT, MULT, accum_out=D[rb][:, :])
            yield
            MDp[(g, w)] = (M, D, p_ps)

        def group_gen(g):
            gens = [diag_gen(g, "c"), diag_gen(g, "s")]
            alive = [True, True]
            while any(alive):
                for i, gi in enumerate(gens):
                    if alive[i]:
                        try:
                            next(gi)
                        except StopIteration:
                            alive[i] = False
                yield
            Mc, Dc, pc_ps = MDp[(g, "c")]
            Ms, Ds, ps_ps = MDp[(g, "s")]
            # comb = 1/sqrt(Ds*Dc); LsT = comb * (Ms A_s) ; T^T = Mc^T LsT
            LsT = [mp.tile([128, 256], bf16, tag=f"LsT{g % 2}{rb}",
                           name=f"LsT{rb}", bufs=1) for rb in range(RB)]
            for rb in range(RB):
                comb = sp.tile([128, 1], f32, tag=f"comb{g % 2}{rb}",
                               name=f"comb{rb}", bufs=1)
                nc.vector.tensor_mul(comb[:, :], Ds[rb][:, :], Dc[rb][:, :])
                nc.scalar.sqrt(comb[:, :], comb[:, :])
                nc.vector.reciprocal(comb[:, :], comb[:, :])
                if rb == 0:
                    nc.vector.tensor_scalar_mul(LsT[rb][:, :],
                                                ps_ps[rb][:, :], comb[:, :])
                else:
                    nc.scalar.mul(LsT[rb][:, :], ps_ps[rb][:, :], comb[:, :])
            yield
            TT = [mp.tile([128, 256], bf16, tag=f"TT{g % 2}{rb}",
                          name=f"TT{rb}", bufs=1) for rb in range(RB)]
            tt_ps = [ppg.tile([128, 256], f32, tag=f"gc{rb}", name="tt")
                     for rb in range(RB)]
            for rb in range(RB):
                for kc in range(RB):
                    mm(tt_ps[rb][:, :], Mc[kc][:, rb * 128:(rb + 1) * 128],
                       LsT[kc][:, :], start=(kc == 0), stop=(kc == RB - 1))
            vcopy(TT[0][:, :], tt_ps[0][:, :])
            scopy(TT[1][:, :], tt_ps[1][:, :])
            yield
            bias = [sp.tile([128, 1], f32, tag=f"bias{g % 2}{rb}",
                            name=f"bias{rb}", bufs=1) for rb in range(RB)]
            for rb in range(RB):
                b_ps = ppw.tile([128, 1], f32, tag="ws", name="b_ps")
                for kc in range(RB):
                    mm(b_ps[:, :], TT[kc][:, rb * 128:(rb + 1) * 128],
                       meanc[g][kc][:, :], start=(kc == 0), stop=(kc == RB - 1))
                nc.vector.tensor_sub(bias[rb][:, :], means[g][rb][:, :],
                                     b_ps[:, :])
            yield
            TTb[g] = (TT, bias)

        aps_tags = ["gc0", "gc1", "gs0", "gs1"]
        aps_ctr = [0]

        def apply_gen(g):
            TT, bias = TTb[g]
            chunks = []
            c0 = 0
            while c0 < Lc:
                cwc = min(ACW, Lc - c0)
                chunks.append((c0, cwc))
                c0 += cwc
            for (c0, cwc) in chunks:
                xn = xnp.tile([128, 2 * ACW], bf16, tag="xn", name="xn")
                for rb in range(RB):
                    nc.sync.dma_start(
                        out=xn[:, rb * cwc:rb * cwc + cwc],
                        in_=xcn[:, g * 2 * Lc + rb * Lc + c0:
                                g * 2 * Lc + rb * Lc + c0 + cwc])
                y2 = yp.tile([128, 2 * ACW], bf16, tag="y2", name="y2")
                nh = cwc // 512
                for rb in range(RB):
                    for h0 in range(0, nh, 2):
                        hs = [h for h in (h0, h0 + 1) if h < nh]
                        pss = []
                        for h in hs:
                            y_ps = ppg.tile(
                                [128, 512], f32,
                                tag=aps_tags[aps_ctr[0] % 4], name="y")
                            aps_ctr[0] += 1
                            pss.append(y_ps)
                        # same stationary weight drives both h-blocks
                        for kc in range(RB):
                            for hi, h in enumerate(hs):
                                mm(pss[hi][:, :],
                                   TT[kc][:, rb * 128:(rb + 1) * 128],
                                   xn[:, kc * cwc + h * 512:
                                      kc * cwc + h * 512 + 512],
                                   start=(kc == 0), stop=(kc == RB - 1))
                        for hi, h in enumerate(hs):
                            dst = y2[:, rb * cwc + h * 512:
                                     rb * cwc + h * 512 + 512]
                            nc.vector.tensor_scalar_add(dst[:, 0:256],
                                                        pss[hi][:, 0:256],
                                                        bias[rb][:, :])
                            nc.scalar.add(dst[:, 256:512],
                                          pss[hi][:, 256:512],
                                          add=bias[rb][:, :])
                        yield
                    # store this rb half as soon as its evacs complete
                    nc.gpsimd.dma_start(
                        out=yout[:, g * 2 * Lc + rb * Lc + c0:
                                 g * 2 * Lc + rb * Lc + c0 + cwc],
                        in_=y2[:, rb * cwc:rb * cwc + cwc])
            yield

        # ---------------- emission schedule ----------------
        for g in range(G):
            for w in ("c", "s"):
                ps = emit_gram(g, w)
                emit_corr(g, w, ps)

        def chain(g):
            yield from group_gen(g)
            yield from apply_gen(g)

        chains = [chain(g) for g in range(G)]
        calive = [True] * G
        # stagger: earlier chains advance twice per later chain's step so
        # group g's apply emission (and scheduler priority) lands ahead of
        # group g+1's diagonalization.
        weights = [2 if g < G - 1 else 1 for g in range(G)]
        while any(calive):
            for i, ci in enumerate(chains):
                for _ in range(weights[i] if any(calive[i + 1:]) else 1):
                    if calive[i]:
                        try:
                            next(ci)
                        except StopIteration:
                            calive[i] = False
    _split_waits(nc)
    return nc


def _consts_np():
    i = np.arange(128)
    f = np.arange(256)
    cstf = np.zeros((128, 1024), dtype=np.float32)
    for rb in range(RB):
        diag = (f[None, :] == (i[:, None] + rb * 128)).astype(np.float32)
        striu = (f[None, :] > (i[:, None] + rb * 128)).astype(np.float32)
        cstf[:, rb * 256:(rb + 1) * 256] = diag - striu
        cstf[:, 512 + rb * 256:512 + (rb + 1) * 256] = diag
    cstb = np.zeros((128, 640), dtype=ml_dtypes.bfloat16)
    for rb in range(RB):
        cstb[:, rb * 256:(rb + 1) * 256] = (
            f[None, :] == (i[:, None] + rb * 128))
    cstb[:, 512:640] = (i[None, :] == i[:, None])
    return cstf, cstb


def kernel(c_feat, s_feat, c_mask, s_mask, _trace=False, _result_box=None):
    from concourse.bass_utils import run_bass_kernel_spmd

    f8 = ml_dtypes.float8_e4m3fn
    bf16 = ml_dtypes.bfloat16

    c_feat = np.ascontiguousarray(np.asarray(c_feat, dtype=np.float32))
    s_feat = np.ascontiguousarray(np.asarray(s_feat, dtype=np.float32))
    cm = np.asarray(c_mask).reshape(-1)
    sm = np.asarray(s_mask).reshape(-1)

    B, Cc, H, W = c_feat.shape
    assert Cc == C
    N = H * W
    cf = c_feat.reshape(B, C, N)
    sf = s_feat.reshape(B, C, N)

    cnt_a = np.bincount(cm, minlength=NUM_LABELS).astype(np.float64)
    cnt_b = np.bincount(sm, minlength=NUM_LABELS).astype(np.float64)
    guide = ((cnt_a > 10) & (cnt_b > 10) & (cnt_a < 10 * cnt_b)
             & (cnt_b < 10 * cnt_a))
    glabels = [l for l in range(NUM_LABELS) if guide[l]]

    out = cf.copy()
    if not glabels:
        return out.reshape(B, C, H, W)

    idx_c = {l: np.nonzero(cm == l)[0] for l in glabels}
    idx_s = {l: np.nonzero(sm == l)[0] for l in glabels}

    groups = [(bb, l) for l in glabels for bb in range(B)]
    n_real = len(groups)
    G = max(1, (n_real + N_CORES - 1) // N_CORES)
    while len(groups) < N_CORES * G:
        groups.append(groups[0])

    def rnd(x, m):
        return ((int(x) + m - 1) // m) * m

    Lc = rnd(max(len(idx_c[l]) for l in glabels), 512)
    Ls = rnd(max(len(idx_s[l]) for l in glabels), 512)
    NCHC = Lc // 128
    NCHS = Ls // 128

    cstf, cstb = _consts_np()
    in_maps = []
    for core in range(N_CORES):
        xc8 = np.zeros((128, G * NCHC * GW), dtype=f8)
        xs8 = np.zeros((128, G * NCHS * GW), dtype=f8)
        xcn = np.zeros((128, G * 2 * Lc), dtype=bf16)
        scal = np.zeros((128, G * 8), dtype=np.float32)
        for g in range(G):
            bb, l = groups[core * G + g]
            ic, isx = idx_c[l], idx_s[l]
            a, b = float(len(ic)), float(len(isx))
            xc = cf[bb][:, ic]                      # [C, a]
            xs = sf[bb][:, isx]
            for (x, L, NCH, dst) in ((xc, Lc, NCHC, xc8), (xs, Ls, NCHS, xs8)):
                n = x.shape[1]
                arr = np.zeros((L, GW), dtype=np.float32)
                arr[:n, 0:128] = x.T[:, 0:128]
                arr[:n, 128] = 1.0
                arr[:n, 129:257] = x.T[:, 128:256]
                t = arr.reshape(NCH, 128, GW).transpose(1, 0, 2)
                dst[:, g * NCH * GW:(g + 1) * NCH * GW] = (
                    t.reshape(128, NCH * GW).astype(f8))
            arr = np.zeros((256, Lc), dtype=np.float32)
            arr[:, :len(ic)] = xc
            xcn[:, g * 2 * Lc:g * 2 * Lc + Lc] = arr[0:128].astype(bf16)
            xcn[:, g * 2 * Lc + Lc:(g + 1) * 2 * Lc] = arr[128:256].astype(bf16)
            vals = [1.0 / (a - 1.0), -1.0 / a, 1.0 / (b - 1.0), -1.0 / b,
                    1.0 / a, 1.0 / b, 0.0, 0.0]
            scal[:, g * 8:(g + 1) * 8] = np.asarray(vals,
                                                    dtype=np.float32)[None, :]
        in_maps.append({"xc8": xc8, "xs8": xs8, "xcn": xcn, "scal": scal,
                        "cstf": cstf, "cstb": cstb})

    nc = _build_graph(G, Lc, Ls)
    res = run_bass_kernel_spmd(nc, in_maps, core_ids=list(range(N_CORES)),
                               trace=_trace)
    if _result_box is not None:
        _result_box.append(res)

    for core in range(N_CORES):
        y = np.asarray(res.results[core]["y"], dtype=np.float32)
        for g in range(G):
            slot = core * G + g
            if slot >= n_real:
                continue
            bb, l = groups[slot]
            ic = idx_c[l]
            na = len(ic)
            out[bb][0:128, ic] = y[:, g * 2 * Lc:g * 2 * Lc + na]
            out[bb][128:256, ic] = y[:, g * 2 * Lc + Lc:g * 2 * Lc + Lc + na]

    return out.reshape(B, C, H, W)
